# revision 27
# baseline (speedup 1.0000x reference)
"""Sigmoid-attention kernel for Trainium2, SPMD over 8 NeuronCores.

Reference computation (per batch b, head h):
    q = (x @ Wq_h) * SCALE ; k = x @ Wk_h ; v = x[:, :, h*64:(h+1)*64]
    out_h = sigmoid((q + bias_h) @ k^T) @ v
Sharding: 8 cores = 4 batches x 2 head-groups (4 heads each).
Each core computes its 4 heads independently; no collectives.

Heads are processed in pairs packed into the two 64-partition halves of
the PE array: head A lives on SBUF partitions 0-63, head B on 64-127.
Scores run as two concurrent 64x128 row-tiles; the P@V matmuls run as
two concurrent 128x64 column-tiles writing the two PSUM halves.

All matmuls contract along SBUF partitions, so x arrives pre-transposed
(features on partitions) and the kernel computes q^T/k^T/S^T/out^T;
the host re-transposes the [dk, n] outputs into the reference layout.

Input DMAs use host-side pre-tiled layouts so every SBUF partition row
is one contiguous 2-8 KB run (big descriptors), split across the two
HWDGE queues in consumption order; SCALE is folded into Wq on the host;
the output DRAM tensor is bf16 (host converts back to f32). A burst of
dependency-free warm-up matmuls on a scratch tile runs during the input
DMA wait so the PE HAM clock-gate is already released (2.4 GHz) when
the first projection starts.
"""
import sys

import numpy as np
import ml_dtypes

try:
    import concourse.bass as bass  # noqa: F401
except ImportError:
    sys.path.insert(0, "/opt/trn_rl_repo")
import concourse.tile as tile
from concourse import bacc, mybir
from concourse.bass_utils import run_bass_kernel_spmd
from concourse.dve_spec import Spec, Src0, One, C0, C1, Bin, AluOp, lower
from concourse.dve_spec import _has_src1 as _has_src1_fn
from concourse.dve_uop import DveOpSpec
from concourse.dve_ops import (
    DveOp, OPS, CUSTOM_DVE_SPECS, _SUB_OPCODE_FOR_NAME, _CUSTOM_DVE_ROW_BASE,
)

BF16 = mybir.dt.bfloat16
F32 = mybir.dt.float32
I32 = mybir.dt.int32
bf16 = ml_dtypes.bfloat16

B, N, DIM = 4, 2048, 512
HEADS, DK = 8, 64
SCALE = DK ** -0.5
NCORES = 8
HPG = 4            # heads per group (= per core)
NPAIR = HPG // 2   # head pairs per core
GD = HPG * DK      # 256: group feature width
DC = DIM // 128    # 4 d-chunks (contraction tiles for projections)
NIC = N // 512     # 4 i-chunks
NJ = N // 128      # 16 j-tiles
NWARM = 10         # PE warm-up matmuls (~4 us cold: releases the HAM gate)

ACT = mybir.ActivationFunctionType
ALU = mybir.AluOpType

# Schraudolph exp constants (exp(-s) ~= bitcast_f32(int32(B - A*s)))
EXP_A = float(2 ** 23 / np.log(2.0))
EXP_B = float(1064867216)

# Custom fused DVE op: out = 1/(1 + in0) via bit-trick reciprocal seed
# (exponent flip: x*bitcast(~x) lands in [-4.5, -4]) + one Newton step.
RC0 = -0.23569351
RC1 = 2.0034004
_SIG_NAME = "SIGMOID_RECIP_TAIL_ANT"


def _ref_sig_tail(in0, in1, c0, c1, c2):
    t = (1.0 + in0).astype(np.float32)
    nt = (~t.view(np.int32)).view(np.float32)
    y0 = (nt * np.float32(c0)).astype(np.float32)
    return y0 * (np.float32(c1) - t * y0)


def _register_sig_tail():
    if _SIG_NAME in _SUB_OPCODE_FOR_NAME:
        return next(o for o in OPS if o.name == _SIG_NAME)
    t = One + Src0
    y0 = Bin(AluOp.BITWISE_NOT, t, t) * C0
    spec = Spec(body=y0 * (C1 - t * y0), reference=_ref_sig_tail)
    opcode = _CUSTOM_DVE_ROW_BASE + len(OPS)
    assert opcode < 0x20
    _SUB_OPCODE_FOR_NAME[_SIG_NAME] = opcode
    shas = {}
    for ver in ("v3", "v4"):
        try:
            sl = DveOpSpec(name=_SIG_NAME, opcode=opcode,
                           uops=lower(spec, ver=ver), rd1_en=_has_src1_fn(spec))
            shas[ver] = sl.sha(ver)
        except Exception:
            pass
    op = DveOp(_SIG_NAME, spec, subdim=False, uops_sha=shas)
    OPS.append(op)
    CUSTOM_DVE_SPECS[_SIG_NAME] = spec
    return op


SIG_OP = _register_sig_tail()


def _build():
    nc = bacc.Bacc("TRN2", target_bir_lowering=False, debug=False)
    # Host-pre-tiled layouts: each declared DRAM tensor's last dim is one
    # contiguous run per SBUF partition (big DMA descriptors).
    #   xt:   [NIC][128][DC*512]  xt[ic][p][dc*512+f] = x[b, ic*512+f, dc*128+p]
    #   wq/wk:[128][DC*GD]        w[p][dc*GD+e]       = W[dc*128+p, e]
    #   v:    [128][NJ*GD]        v[p][jc*GD+e]       = x[b, jc*128+p, gs][e]
    xt_d = nc.declare_dram_parameter("xt", [NIC, 128, DC * 512], BF16,
                                     isOutput=False)
    wq = nc.declare_dram_parameter("wq", [128, DC * GD], BF16, isOutput=False)
    wk = nc.declare_dram_parameter("wk", [128, DC * GD], BF16, isOutput=False)
    vv = nc.declare_dram_parameter("v", [128, NJ * GD], BF16, isOutput=False)
    bias = nc.declare_dram_parameter("bias", [128, NPAIR], F32, isOutput=False)
    out = nc.declare_dram_parameter("out", [NPAIR, 128, N], BF16, isOutput=True)

    with tile.TileContext(nc) as tc:
        with (
            tc.tile_pool(name="const", bufs=1) as cpool,
            tc.tile_pool(name="qk", bufs=8) as qkpool,
            tc.tile_pool(name="pp", bufs=6) as ppool,
            tc.tile_pool(name="dve", bufs=2) as dvepool,
            tc.tile_pool(name="osb", bufs=2) as opool,
            tc.tile_pool(name="ps_proj", bufs=1, space="PSUM") as pjpool,
            tc.tile_pool(name="ps_s", bufs=2, space="PSUM") as spool,
            tc.tile_pool(name="ps_sd", bufs=1, space="PSUM") as sdpool,
            tc.tile_pool(name="ps_o", bufs=1, space="PSUM") as oppool,
        ):
            # ---- PE warm-up: dependency-free matmuls on a memset scratch
            # tile, running during the input-DMA wait. The HAM clock gate
            # needs ~3.4 us of sustained PE activity to go 4/8 -> 8/8.
            scr = cpool.tile([128, 512], BF16, name="scratch")
            nc.gpsimd.memset(scr[:], 0.0)
            wps = pjpool.tile([128, 512], F32, tag="pj", name="warm")
            for _ in range(NWARM):
                nc.tensor.matmul(wps[:], scr[:, 0:128], scr[:],
                                 start=True, stop=True)

            # ---- constants. Loads are spread over the two HWDGE queues in
            # consumption order; sync carries the critical path (wq, x ics).
            # xt ic0 split in half so the first projection matmuls start
            # after 256 KB instead of 512 KB
            xt0h = [cpool.tile([128, 1024], BF16, name=f"xt0{h}")
                    for h in range(2)]
            xt_t = [None] + [cpool.tile([128, DC * 512], BF16, name=f"xt{ic}")
                             for ic in range(1, NIC)]
            nc.sync.dma_start(xt0h[0][:], xt_d[0][:, 0:1024])
            wq_t = cpool.tile([128, DC * GD], BF16, name="wqt")
            nc.sync.dma_start(wq_t[:], wq[:, :])
            nc.sync.dma_start(xt0h[1][:], xt_d[0][:, 1024:2048])
            nc.sync.dma_start(xt_t[1][:], xt_d[1])
            bt = cpool.tile([128, NPAIR], F32, name="biast")
            nc.scalar.dma_start(bt[:], bias[:, :])
            bias_t = [bt[:, p:p + 1] for p in range(NPAIR)]
            wk_t = cpool.tile([128, DC * GD], BF16, name="wkt")
            nc.scalar.dma_start(wk_t[:], wk[:, :])
            v_t = cpool.tile([128, NJ * GD], BF16, name="vt")
            nc.scalar.dma_start(v_t[:], vv[:, :])
            nc.scalar.dma_start(xt_t[2][:], xt_d[2])
            nc.scalar.dma_start(xt_t[3][:], xt_d[3])

            qbT_all, kT_all = {}, {}

            qkpart = {}

            def emit_proj(p, ic, k_first=False, aux="vector"):
                if k_first == "q":
                    qbT, kT = qkpart[(p, ic)], kT_all[(p, ic)]
                else:
                    qbT = qkpool.tile([128, 512], BF16, tag="qbT",
                                      name=f"qbT{p}_{ic}")
                    kT = qkpool.tile([128, 512], BF16, tag="kT",
                                     name=f"kT{p}_{ic}")

                def xsl(dc):
                    if ic == 0:
                        return xt0h[dc // 2][:, (dc % 2) * 512:(dc % 2 + 1) * 512]
                    return xt_t[ic][:, dc * 512:(dc + 1) * 512]

                def chain(w_t, ps):
                    for dc in range(DC):
                        nc.tensor.matmul(
                            ps[:],
                            w_t[:, dc * GD + p * 128: dc * GD + (p + 1) * 128],
                            xsl(dc),
                            start=(dc == 0), stop=(dc == DC - 1),
                        )

                def do_q():
                    pq = pjpool.tile([128, 512], F32, tag="pj",
                                     name=f"pq{p}_{ic}")
                    chain(wq_t, pq)
                    # qb = q + bias (per-partition; SCALE folded into Wq on
                    # host), cast to bf16
                    if aux == "scalar":
                        nc.scalar.activation(qbT[:], pq[:], ACT.Identity,
                                             bias=bias_t[p])
                    else:
                        nc.vector.tensor_scalar_add(qbT[:], pq[:],
                                                    bias_t[p])

                def do_k():
                    pk = pjpool.tile([128, 512], F32, tag="pj",
                                     name=f"pk{p}_{ic}")
                    chain(wk_t, pk)
                    if aux == "scalar":
                        nc.scalar.copy(kT[:], pk[:])
                    else:
                        nc.vector.tensor_copy(kT[:], pk[:])

                if k_first == "k":
                    do_k()
                    kT_all[(p, ic)] = kT
                    qkpart[(p, ic)] = qbT
                elif k_first == "q":
                    qbT = qkpart[(p, ic)]
                    do_q()
                    qbT_all[(p, ic)] = qbT
                elif k_first:
                    do_k(), do_q()
                    qbT_all[(p, ic)] = qbT
                    kT_all[(p, ic)] = kT
                else:
                    do_q(), do_k()
                    qbT_all[(p, ic)] = qbT
                    kT_all[(p, ic)] = kT

            def scores(p, j, ic, s_ps):
                kslc = kT_all[(p, j // 4)][:, (j % 4) * 128:(j % 4 + 1) * 128]
                # two concurrent 64x128 row-tiles (head A rows 0-63, B 64-127)
                nc.tensor.matmul(
                    s_ps[:, 0:512], kslc[0:64, :], qbT_all[(p, ic)][0:64, :],
                    start=True, stop=True,
                )
                nc.tensor.matmul(
                    s_ps[:, 512:1024], kslc[64:128, :],
                    qbT_all[(p, ic)][64:128, :],
                    start=True, stop=True,
                )

            def pv(p, j, o_ps, p_sb, start, stop):
                ha, hb = 2 * p, 2 * p + 1
                # P @ v: two concurrent 128x64 col-tiles into PSUM halves
                nc.tensor.matmul(
                    o_ps[0:64, :],
                    v_t[:, j * GD + ha * DK: j * GD + (ha + 1) * DK],
                    p_sb[:, 0:512],
                    start=start, stop=stop,
                )
                nc.tensor.matmul(
                    o_ps[64:128, :],
                    v_t[:, j * GD + hb * DK: j * GD + (hb + 1) * DK],
                    p_sb[:, 512:1024],
                    start=start, stop=stop,
                )

            def launch_dve(p, j, ic):
                # sigmoid = 1/(1 + schraudolph_exp(-s)) on VectorE; its own
                # PSUM tile so it never blocks the ScalarE lane's buffers
                s_ps = sdpool.tile([128, 1024], F32, tag="sd",
                                   name=f"sd{p}_{ic}_{j}")
                scores(p, j, ic, s_ps)
                it = dvepool.tile([128, 1024], I32, tag="sit",
                                  name=f"sit{p}_{ic}_{j}")
                nc.vector.tensor_scalar(it[:], s_ps[:], -EXP_A, EXP_B,
                                        ALU.mult, ALU.add)
                p_sb = ppool.tile([128, 1024], BF16, tag="pgd",
                                  name=f"prd{p}_{ic}_{j}")
                nc.vector._custom_dve(SIG_OP, out=p_sb[:],
                                      in0=it[:].bitcast(F32), s0=RC0, s1=RC1)
                return p_sb

            # j-groups per window on VectorE; extra share around the pair
            # transition and on the final window (both lanes drain together
            # instead of a serial ScalarE tail)
            NDVE_PAT = [4, 5, 5, 5, 5, 4, 5, 5]
            out_sb_t = {}

            def window(p, ic, inserts=(), ops_pool=None):
                if ic == 0:
                    out_sb_t[p] = opool.tile([128, N], BF16, tag="osb",
                                             name=f"osb{p}")
                out_sb = out_sb_t[p]
                ndve = NDVE_PAT[p * NIC + ic]
                last_w = (p == NPAIR - 1 and ic == NIC - 1)
                if (p == 0 and ic == 0) or last_w:
                    # window 0: DVE j's low so their scores only need the
                    # first kT tiles (kT2/kT3 are still being projected).
                    # final window: low j's + inline PVs so the DVE lane
                    # finishes with ScalarE instead of a serial PE tail.
                    dve_js = [2 * u + 1 for u in range(ndve)]
                else:
                    dve_js = list(range(NJ - ndve, NJ))
                sc_js = [j for j in range(NJ) if j not in dve_js]
                # pair-1 windows alternate the accumulator between the
                # ops bank and the (by then idle) proj bank, so window w's
                # out-copy never serializes against window w+1's first PV
                pool = ops_pool or oppool
                tag = "ops" if pool is oppool else "pj"
                o_ps = pool.tile([128, 512], F32, tag=tag,
                                 name=f"ops{p}_{ic}")
                dve_p = {}
                dve_done = set()
                dve_p[dve_js[0]] = launch_dve(p, dve_js[0], ic)
                prev = None  # PV deferred one step: scores(j+1) lands first
                for idx, j in enumerate(sc_js):
                    s_ps = spool.tile([128, 1024], F32, tag="sg",
                                      name=f"s{p}_{ic}_{j}")
                    scores(p, j, ic, s_ps)
                    p_sb = ppool.tile([128, 1024], BF16, tag="pg",
                                      name=f"pr{p}_{ic}_{j}")
                    nc.scalar.activation(p_sb[:], s_ps[:], ACT.Sigmoid)
                    if prev is not None:
                        pv(p, prev[0], o_ps, prev[1], start=(idx == 1),
                           stop=False)
                    prev = (j, p_sb)
                    for step, jd in enumerate(dve_js[1:]):
                        if idx == 2 + 2 * step:
                            dve_p[jd] = launch_dve(p, jd, ic)
                    if last_w:
                        # dve PV for jd launched at idx-3 (p_sb ready ~2.5us
                        # after its scores; 3 Sc tiles =~ 3.4us of slack)
                        for step, jd in enumerate(dve_js):
                            if idx == 4 + 2 * step:
                                pv(p, jd, o_ps, dve_p[jd], start=False,
                                   stop=False)
                                dve_done.add(jd)
                    for at, pp, pic, kf, aux in inserts:
                        if idx == at:
                            emit_proj(pp, pic, k_first=kf, aux=aux)
                rest = [j for j in dve_js if j not in dve_done]
                if rest:
                    pv(p, prev[0], o_ps, prev[1], start=False, stop=False)
                    for j in rest:
                        pv(p, j, o_ps, dve_p[j], start=False,
                           stop=(j == rest[-1]))
                else:
                    pv(p, prev[0], o_ps, prev[1], start=False, stop=True)
                oslc = out_sb[:, ic * 512:(ic + 1) * 512]
                if last_w or (p == 0 and ic <= 2):
                    # ScalarE idles ~1-2us at these window boundaries anyway
                    # and executes the drain immediately (VectorE's backlog
                    # would hold the o_ps bank and stall the next window's
                    # first PV on the in-order PE)
                    nc.scalar.copy(oslc, o_ps[:])
                else:
                    nc.vector.tensor_copy(oslc, o_ps[:])
                nc.sync.dma_start(out[p][:, ic * 512:(ic + 1) * 512], oslc)

            # Only proj(0,0) runs up front; every other projection chain
            # is dripped into a window's PE slack (k-chain first, placed just
            # before the j-sweep needs that kT), so no projection burst ever
            # starves ScalarE.
            emit_proj(0, 0, aux="scalar")
            window(0, 0, inserts=((0, 0, 1, "k", "vector"),
                                  (2, 0, 2, "k", "vector"),
                                  (4, 0, 3, "k", "vector"),
                                  (7, 0, 1, "q", "vector"),
                                  (9, 0, 2, "q", "vector"),
                                  (11, 0, 3, "q", "vector")))
            window(0, 1, inserts=((2, 1, 0, "k", "vector"),
                                  (4, 1, 0, "q", "vector"),
                                  (6, 1, 1, "k", "vector"),
                                  (8, 1, 1, "q", "vector")))
            window(0, 2, inserts=((2, 1, 2, "k", "vector"),
                                  (4, 1, 2, "q", "vector"),
                                  (6, 1, 3, "k", "vector"),
                                  (8, 1, 3, "q", "vector")))
            # all chains done by end of (0,2): the proj bank joins the o_ps
            # alternation one window earlier, decoupling the (0,2)->(0,3)
            # and (0,3)->(1,0) boundaries as well
            window(0, 3, ops_pool=pjpool)
            window(1, 0)
            window(1, 1, ops_pool=pjpool)
            window(1, 2)
            window(1, 3, ops_pool=pjpool)
    nc.compile()
    return nc


_NC_CACHE = None


def _get_nc():
    global _NC_CACHE
    if _NC_CACHE is None:
        _NC_CACHE = _build()
    return _NC_CACHE


def _make_in_maps(x, Wq, Wk, rb):
    # xt[ic][p][dc*512+f] = xT[dc*128+p, ic*512+f] = x[b, ic*512+f, dc*128+p]
    xt_b = []
    for b in range(B):
        xT = np.ascontiguousarray(x[b].T).astype(bf16)          # [512, 2048]
        xt = xT.reshape(DC, 128, NIC, 512).transpose(2, 1, 0, 3)
        xt_b.append(np.ascontiguousarray(xt.reshape(NIC, 128, DC * 512)))
    wq_s = (Wq * SCALE).astype(bf16)   # SCALE folded into Wq
    wk_bf = Wk.astype(bf16)
    bias_flat = rb.reshape(HEADS * DK, 1)  # [512, 1] head-major

    in_maps = []
    for c in range(NCORES):
        b, g = divmod(c, 2)
        gs = slice(g * GD, (g + 1) * GD)
        # w[p][dc*GD+e] = W[dc*128+p, gs][e]
        wq_t = wq_s[:, gs].reshape(DC, 128, GD).transpose(1, 0, 2)
        wk_t = wk_bf[:, gs].reshape(DC, 128, GD).transpose(1, 0, 2)
        # v[p][jc*GD+e] = x[b, jc*128+p, gs][e]
        v_t = x[b, :, gs].astype(bf16).reshape(NJ, 128, GD).transpose(1, 0, 2)
        bias_g = bias_flat[g * GD:(g + 1) * GD].reshape(NPAIR, 128).T
        in_maps.append({
            "xt": xt_b[b],
            "wq": np.ascontiguousarray(wq_t.reshape(128, DC * GD)),
            "wk": np.ascontiguousarray(wk_t.reshape(128, DC * GD)),
            "v": np.ascontiguousarray(v_t.reshape(128, NJ * GD)),
            "bias": np.ascontiguousarray(bias_g),
        })
    return in_maps


def _gather(results):
    out_full = np.empty((B, N, DIM), dtype=np.float32)
    for c in range(NCORES):
        b, g = divmod(c, 2)
        oc = results[c]["out"].astype(np.float32)  # [NPAIR, 128, N] bf16
        for p in range(NPAIR):
            for u in range(2):
                h = 2 * p + u
                col = g * GD + h * DK
                out_full[b, :, col:col + DK] = oc[p, u * 64:(u + 1) * 64, :].T
    return out_full


def kernel(x, Wq, Wk, rel_content_bias):
    x = np.asarray(x, dtype=np.float32)
    Wq = np.asarray(Wq, dtype=np.float32)
    Wk = np.asarray(Wk, dtype=np.float32)
    rb = np.asarray(rel_content_bias, dtype=np.float32)

    nc = _get_nc()
    in_maps = _make_in_maps(x, Wq, Wk, rb)
    res = run_bass_kernel_spmd(nc, in_maps, core_ids=list(range(NCORES)))
    return _gather(res.results)


# revision 28
# speedup vs baseline: 1.0313x; 1.0313x over previous
"""Sigmoid-attention kernel for Trainium2, SPMD over 8 NeuronCores.

Reference computation (per batch b, head h):
    q = (x @ Wq_h) * SCALE ; k = x @ Wk_h ; v = x[:, :, h*64:(h+1)*64]
    out_h = sigmoid((q + bias_h) @ k^T) @ v
Sharding: 8 cores = 4 batches x 2 head-groups (4 heads each).
Each core computes its 4 heads independently; no collectives.

Heads are processed in pairs packed into the two 64-partition halves of
the PE array: head A lives on SBUF partitions 0-63, head B on 64-127.
Scores run as two concurrent 64x128 row-tiles; the P@V matmuls run as
two concurrent 128x64 column-tiles writing the two PSUM halves.

All matmuls contract along SBUF partitions, so x arrives pre-transposed
(features on partitions) and the kernel computes q^T/k^T/S^T/out^T;
the host re-transposes the [dk, n] outputs into the reference layout.

Input DMAs use host-side pre-tiled layouts so every SBUF partition row
is one contiguous 2-8 KB run (big descriptors), split across the two
HWDGE queues in consumption order; SCALE is folded into Wq on the host;
the output DRAM tensor is bf16 (host converts back to f32). A burst of
dependency-free warm-up matmuls on a scratch tile runs during the input
DMA wait so the PE HAM clock-gate is already released (2.4 GHz) when
the first projection starts.
"""
import sys

import numpy as np
import ml_dtypes

try:
    import concourse.bass as bass  # noqa: F401
except ImportError:
    sys.path.insert(0, "/opt/trn_rl_repo")
import concourse.tile as tile
from concourse import bacc, mybir
from concourse.bass_utils import run_bass_kernel_spmd
from concourse.dve_spec import Spec, Src0, One, C0, C1, Bin, AluOp, lower
from concourse.dve_spec import _has_src1 as _has_src1_fn
from concourse.dve_uop import DveOpSpec
from concourse.dve_ops import (
    DveOp, OPS, CUSTOM_DVE_SPECS, _SUB_OPCODE_FOR_NAME, _CUSTOM_DVE_ROW_BASE,
)

BF16 = mybir.dt.bfloat16
F32 = mybir.dt.float32
I32 = mybir.dt.int32
bf16 = ml_dtypes.bfloat16

B, N, DIM = 4, 2048, 512
HEADS, DK = 8, 64
SCALE = DK ** -0.5
NCORES = 8
HPG = 4            # heads per group (= per core)
NPAIR = HPG // 2   # head pairs per core
GD = HPG * DK      # 256: group feature width
DC = DIM // 128    # 4 d-chunks (contraction tiles for projections)
NIC = N // 512     # 4 i-chunks
NJ = N // 128      # 16 j-tiles
NWARM = 10         # PE warm-up matmuls (~4 us cold: releases the HAM gate)

ACT = mybir.ActivationFunctionType
ALU = mybir.AluOpType

# Schraudolph exp constants (exp(-s) ~= bitcast_f32(int32(B - A*s)))
EXP_A = float(2 ** 23 / np.log(2.0))
EXP_B = float(1064867216)

# Custom fused DVE op: out = 1/(1 + in0) via bit-trick reciprocal seed
# (exponent flip: x*bitcast(~x) lands in [-4.5, -4]) + one Newton step.
RC0 = -0.23569351
RC1 = 2.0034004
_SIG_NAME = "SIGMOID_RECIP_TAIL_ANT"


def _ref_sig_tail(in0, in1, c0, c1, c2):
    t = (1.0 + in0).astype(np.float32)
    nt = (~t.view(np.int32)).view(np.float32)
    y0 = (nt * np.float32(c0)).astype(np.float32)
    return y0 * (np.float32(c1) - t * y0)


def _register_sig_tail():
    if _SIG_NAME in _SUB_OPCODE_FOR_NAME:
        return next(o for o in OPS if o.name == _SIG_NAME)
    t = One + Src0
    y0 = Bin(AluOp.BITWISE_NOT, t, t) * C0
    spec = Spec(body=y0 * (C1 - t * y0), reference=_ref_sig_tail)
    opcode = _CUSTOM_DVE_ROW_BASE + len(OPS)
    assert opcode < 0x20
    _SUB_OPCODE_FOR_NAME[_SIG_NAME] = opcode
    shas = {}
    for ver in ("v3", "v4"):
        try:
            sl = DveOpSpec(name=_SIG_NAME, opcode=opcode,
                           uops=lower(spec, ver=ver), rd1_en=_has_src1_fn(spec))
            shas[ver] = sl.sha(ver)
        except Exception:
            pass
    op = DveOp(_SIG_NAME, spec, subdim=False, uops_sha=shas)
    OPS.append(op)
    CUSTOM_DVE_SPECS[_SIG_NAME] = spec
    return op


SIG_OP = _register_sig_tail()


def _build():
    nc = bacc.Bacc("TRN2", target_bir_lowering=False, debug=False)
    # Host-pre-tiled layouts: each declared DRAM tensor's last dim is one
    # contiguous run per SBUF partition (big DMA descriptors).
    #   xt:   [NIC][128][DC*512]  xt[ic][p][dc*512+f] = x[b, ic*512+f, dc*128+p]
    #   wq/wk:[128][DC*GD]        w[p][dc*GD+e]       = W[dc*128+p, e]
    #   v:    [128][NJ*GD]        v[p][jc*GD+e]       = x[b, jc*128+p, gs][e]
    xt_d = nc.declare_dram_parameter("xt", [NIC, 128, DC * 512], BF16,
                                     isOutput=False)
    wq = nc.declare_dram_parameter("wq", [128, DC * GD], BF16, isOutput=False)
    wk = nc.declare_dram_parameter("wk", [128, DC * GD], BF16, isOutput=False)
    vv = nc.declare_dram_parameter("v", [128, NJ * GD], BF16, isOutput=False)
    bias = nc.declare_dram_parameter("bias", [128, NPAIR], F32, isOutput=False)
    out = nc.declare_dram_parameter("out", [NPAIR, 128, N], BF16, isOutput=True)

    with tile.TileContext(nc) as tc:
        with (
            tc.tile_pool(name="const", bufs=1) as cpool,
            tc.tile_pool(name="qk", bufs=8) as qkpool,
            tc.tile_pool(name="pp", bufs=6) as ppool,
            tc.tile_pool(name="dve", bufs=2) as dvepool,
            tc.tile_pool(name="osb", bufs=2) as opool,
            tc.tile_pool(name="ps_proj", bufs=1, space="PSUM") as pjpool,
            tc.tile_pool(name="ps_s", bufs=2, space="PSUM") as spool,
            tc.tile_pool(name="ps_sd", bufs=1, space="PSUM") as sdpool,
            tc.tile_pool(name="ps_o", bufs=1, space="PSUM") as oppool,
        ):
            # ---- PE warm-up: dependency-free matmuls on a memset scratch
            # tile, running during the input-DMA wait. The HAM clock gate
            # needs ~3.4 us of sustained PE activity to go 4/8 -> 8/8.
            scr = cpool.tile([128, 512], BF16, name="scratch")
            nc.gpsimd.memset(scr[:], 0.0)
            wps = pjpool.tile([128, 512], F32, tag="pj", name="warm")
            for _ in range(NWARM):
                nc.tensor.matmul(wps[:], scr[:, 0:128], scr[:],
                                 start=True, stop=True)

            # ---- constants. Loads are spread over the two HWDGE queues in
            # consumption order; sync carries the critical path (wq, x ics).
            # xt ic0 split in half so the first projection matmuls start
            # after 256 KB instead of 512 KB
            xt0h = [cpool.tile([128, 1024], BF16, name=f"xt0{h}")
                    for h in range(2)]
            xt_t = [None] + [cpool.tile([128, DC * 512], BF16, name=f"xt{ic}")
                             for ic in range(1, NIC)]
            nc.sync.dma_start(xt0h[0][:], xt_d[0][:, 0:1024])
            wq_t = cpool.tile([128, DC * GD], BF16, name="wqt")
            nc.sync.dma_start(wq_t[:], wq[:, :])
            nc.sync.dma_start(xt0h[1][:], xt_d[0][:, 1024:2048])
            nc.sync.dma_start(xt_t[1][:], xt_d[1])
            wk_t = cpool.tile([128, DC * GD], BF16, name="wkt")
            nc.scalar.dma_start(wk_t[:], wk[:, :])
            bt = cpool.tile([128, NPAIR], F32, name="biast")
            nc.scalar.dma_start(bt[:], bias[:, :])
            bias_t = [bt[:, p:p + 1] for p in range(NPAIR)]
            v_t = cpool.tile([128, NJ * GD], BF16, name="vt")
            nc.scalar.dma_start(v_t[:], vv[:, :])
            nc.scalar.dma_start(xt_t[2][:], xt_d[2])
            nc.scalar.dma_start(xt_t[3][:], xt_d[3])

            qbT_all, kT_all = {}, {}

            qkpart = {}

            def emit_proj(p, ic, k_first=False, aux="vector"):
                if k_first == "q":
                    qbT, kT = qkpart[(p, ic)], kT_all[(p, ic)]
                else:
                    qbT = qkpool.tile([128, 512], BF16, tag="qbT",
                                      name=f"qbT{p}_{ic}")
                    kT = qkpool.tile([128, 512], BF16, tag="kT",
                                     name=f"kT{p}_{ic}")

                def xsl(dc):
                    if ic == 0:
                        return xt0h[dc // 2][:, (dc % 2) * 512:(dc % 2 + 1) * 512]
                    return xt_t[ic][:, dc * 512:(dc + 1) * 512]

                def chain(w_t, ps):
                    for dc in range(DC):
                        nc.tensor.matmul(
                            ps[:],
                            w_t[:, dc * GD + p * 128: dc * GD + (p + 1) * 128],
                            xsl(dc),
                            start=(dc == 0), stop=(dc == DC - 1),
                        )

                def do_q():
                    pq = pjpool.tile([128, 512], F32, tag="pj",
                                     name=f"pq{p}_{ic}")
                    chain(wq_t, pq)
                    # qb = q + bias (per-partition; SCALE folded into Wq on
                    # host), cast to bf16
                    if aux == "scalar":
                        nc.scalar.activation(qbT[:], pq[:], ACT.Identity,
                                             bias=bias_t[p])
                    else:
                        nc.vector.tensor_scalar_add(qbT[:], pq[:],
                                                    bias_t[p])

                def do_k():
                    pk = pjpool.tile([128, 512], F32, tag="pj",
                                     name=f"pk{p}_{ic}")
                    chain(wk_t, pk)
                    if aux == "scalar":
                        nc.scalar.copy(kT[:], pk[:])
                    else:
                        nc.vector.tensor_copy(kT[:], pk[:])

                if k_first == "k":
                    do_k()
                    kT_all[(p, ic)] = kT
                    qkpart[(p, ic)] = qbT
                elif k_first == "q":
                    qbT = qkpart[(p, ic)]
                    do_q()
                    qbT_all[(p, ic)] = qbT
                elif k_first:
                    do_k(), do_q()
                    qbT_all[(p, ic)] = qbT
                    kT_all[(p, ic)] = kT
                else:
                    do_q(), do_k()
                    qbT_all[(p, ic)] = qbT
                    kT_all[(p, ic)] = kT

            def scores(p, j, ic, s_ps):
                kslc = kT_all[(p, j // 4)][:, (j % 4) * 128:(j % 4 + 1) * 128]
                # two concurrent 64x128 row-tiles (head A rows 0-63, B 64-127)
                nc.tensor.matmul(
                    s_ps[:, 0:512], kslc[0:64, :], qbT_all[(p, ic)][0:64, :],
                    start=True, stop=True,
                )
                nc.tensor.matmul(
                    s_ps[:, 512:1024], kslc[64:128, :],
                    qbT_all[(p, ic)][64:128, :],
                    start=True, stop=True,
                )

            def pv(p, j, o_ps, p_sb, start, stop):
                ha, hb = 2 * p, 2 * p + 1
                # P @ v: two concurrent 128x64 col-tiles into PSUM halves
                nc.tensor.matmul(
                    o_ps[0:64, :],
                    v_t[:, j * GD + ha * DK: j * GD + (ha + 1) * DK],
                    p_sb[:, 0:512],
                    start=start, stop=stop,
                )
                nc.tensor.matmul(
                    o_ps[64:128, :],
                    v_t[:, j * GD + hb * DK: j * GD + (hb + 1) * DK],
                    p_sb[:, 512:1024],
                    start=start, stop=stop,
                )

            def launch_dve(p, j, ic):
                # sigmoid = 1/(1 + schraudolph_exp(-s)) on VectorE; its own
                # PSUM tile so it never blocks the ScalarE lane's buffers
                s_ps = sdpool.tile([128, 1024], F32, tag="sd",
                                   name=f"sd{p}_{ic}_{j}")
                scores(p, j, ic, s_ps)
                it = dvepool.tile([128, 1024], I32, tag="sit",
                                  name=f"sit{p}_{ic}_{j}")
                nc.vector.tensor_scalar(it[:], s_ps[:], -EXP_A, EXP_B,
                                        ALU.mult, ALU.add)
                p_sb = ppool.tile([128, 1024], BF16, tag="pgd",
                                  name=f"prd{p}_{ic}_{j}")
                nc.vector._custom_dve(SIG_OP, out=p_sb[:],
                                      in0=it[:].bitcast(F32), s0=RC0, s1=RC1)
                return p_sb

            # j-groups per window on VectorE; extra share around the pair
            # transition and on the final window (both lanes drain together
            # instead of a serial ScalarE tail)
            NDVE_PAT = [4, 5, 5, 5, 5, 4, 5, 5]
            out_sb_t = {}

            def window(p, ic, inserts=(), ops_pool=None):
                if ic == 0:
                    out_sb_t[p] = opool.tile([128, N], BF16, tag="osb",
                                             name=f"osb{p}")
                out_sb = out_sb_t[p]
                ndve = NDVE_PAT[p * NIC + ic]
                last_w = (p == NPAIR - 1 and ic == NIC - 1)
                if (p == 0 and ic == 0) or last_w:
                    # window 0: DVE j's low so their scores only need the
                    # first kT tiles (kT2/kT3 are still being projected).
                    # final window: low j's + inline PVs so the DVE lane
                    # finishes with ScalarE instead of a serial PE tail.
                    dve_js = [2 * u + 1 for u in range(ndve)]
                else:
                    dve_js = list(range(NJ - ndve, NJ))
                sc_js = [j for j in range(NJ) if j not in dve_js]
                # pair-1 windows alternate the accumulator between the
                # ops bank and the (by then idle) proj bank, so window w's
                # out-copy never serializes against window w+1's first PV
                pool = ops_pool or oppool
                tag = "ops" if pool is oppool else "pj"
                o_ps = pool.tile([128, 512], F32, tag=tag,
                                 name=f"ops{p}_{ic}")
                dve_p = {}
                dve_done = set()
                dve_p[dve_js[0]] = launch_dve(p, dve_js[0], ic)
                prev = None  # PV deferred one step: scores(j+1) lands first
                for idx, j in enumerate(sc_js):
                    s_ps = spool.tile([128, 1024], F32, tag="sg",
                                      name=f"s{p}_{ic}_{j}")
                    scores(p, j, ic, s_ps)
                    p_sb = ppool.tile([128, 1024], BF16, tag="pg",
                                      name=f"pr{p}_{ic}_{j}")
                    nc.scalar.activation(p_sb[:], s_ps[:], ACT.Sigmoid)
                    if prev is not None:
                        pv(p, prev[0], o_ps, prev[1], start=(idx == 1),
                           stop=False)
                    prev = (j, p_sb)
                    for step, jd in enumerate(dve_js[1:]):
                        if idx == 2 + 2 * step:
                            dve_p[jd] = launch_dve(p, jd, ic)
                    if last_w:
                        # dve PV for jd launched at idx-3 (p_sb ready ~2.5us
                        # after its scores; 3 Sc tiles =~ 3.4us of slack)
                        for step, jd in enumerate(dve_js):
                            if idx == 4 + 2 * step:
                                pv(p, jd, o_ps, dve_p[jd], start=False,
                                   stop=False)
                                dve_done.add(jd)
                    for at, pp, pic, kf, aux in inserts:
                        if idx == at:
                            emit_proj(pp, pic, k_first=kf, aux=aux)
                rest = [j for j in dve_js if j not in dve_done]
                if rest:
                    pv(p, prev[0], o_ps, prev[1], start=False, stop=False)
                    for j in rest:
                        pv(p, j, o_ps, dve_p[j], start=False,
                           stop=(j == rest[-1]))
                else:
                    pv(p, prev[0], o_ps, prev[1], start=False, stop=True)
                oslc = out_sb[:, ic * 512:(ic + 1) * 512]
                if last_w or (p == 0 and ic <= 2):
                    # ScalarE idles ~1-2us at these window boundaries anyway
                    # and executes the drain immediately (VectorE's backlog
                    # would hold the o_ps bank and stall the next window's
                    # first PV on the in-order PE)
                    nc.scalar.copy(oslc, o_ps[:])
                else:
                    nc.vector.tensor_copy(oslc, o_ps[:])
                nc.sync.dma_start(out[p][:, ic * 512:(ic + 1) * 512], oslc)

            # Only proj(0,0) runs up front; every other projection chain
            # is dripped into a window's PE slack (k-chain first, placed just
            # before the j-sweep needs that kT), so no projection burst ever
            # starves ScalarE.
            emit_proj(0, 0, aux="scalar")
            window(0, 0, inserts=((0, 0, 1, "k", "vector"),
                                  (2, 0, 2, "k", "vector"),
                                  (4, 0, 3, "k", "vector"),
                                  (7, 0, 1, "q", "vector"),
                                  (9, 0, 2, "q", "vector"),
                                  (11, 0, 3, "q", "vector")))
            window(0, 1, inserts=((2, 1, 0, "k", "vector"),
                                  (4, 1, 0, "q", "vector"),
                                  (6, 1, 1, "k", "vector"),
                                  (8, 1, 1, "q", "vector")))
            window(0, 2, inserts=((2, 1, 2, "k", "vector"),
                                  (4, 1, 2, "q", "vector"),
                                  (6, 1, 3, "k", "vector"),
                                  (8, 1, 3, "q", "vector")))
            # all chains done by end of (0,2): the proj bank joins the o_ps
            # alternation one window earlier, decoupling the (0,2)->(0,3)
            # and (0,3)->(1,0) boundaries as well
            window(0, 3, ops_pool=pjpool)
            window(1, 0)
            window(1, 1, ops_pool=pjpool)
            window(1, 2)
            window(1, 3, ops_pool=pjpool)
    nc.compile()
    return nc


_NC_CACHE = None


def _get_nc():
    global _NC_CACHE
    if _NC_CACHE is None:
        _NC_CACHE = _build()
    return _NC_CACHE


def _make_in_maps(x, Wq, Wk, rb):
    # xt[ic][p][dc*512+f] = xT[dc*128+p, ic*512+f] = x[b, ic*512+f, dc*128+p]
    xt_b = []
    for b in range(B):
        xT = np.ascontiguousarray(x[b].T).astype(bf16)          # [512, 2048]
        xt = xT.reshape(DC, 128, NIC, 512).transpose(2, 1, 0, 3)
        xt_b.append(np.ascontiguousarray(xt.reshape(NIC, 128, DC * 512)))
    wq_s = (Wq * SCALE).astype(bf16)   # SCALE folded into Wq
    wk_bf = Wk.astype(bf16)
    bias_flat = rb.reshape(HEADS * DK, 1)  # [512, 1] head-major

    in_maps = []
    for c in range(NCORES):
        b, g = divmod(c, 2)
        gs = slice(g * GD, (g + 1) * GD)
        # w[p][dc*GD+e] = W[dc*128+p, gs][e]
        wq_t = wq_s[:, gs].reshape(DC, 128, GD).transpose(1, 0, 2)
        wk_t = wk_bf[:, gs].reshape(DC, 128, GD).transpose(1, 0, 2)
        # v[p][jc*GD+e] = x[b, jc*128+p, gs][e]
        v_t = x[b, :, gs].astype(bf16).reshape(NJ, 128, GD).transpose(1, 0, 2)
        bias_g = bias_flat[g * GD:(g + 1) * GD].reshape(NPAIR, 128).T
        in_maps.append({
            "xt": xt_b[b],
            "wq": np.ascontiguousarray(wq_t.reshape(128, DC * GD)),
            "wk": np.ascontiguousarray(wk_t.reshape(128, DC * GD)),
            "v": np.ascontiguousarray(v_t.reshape(128, NJ * GD)),
            "bias": np.ascontiguousarray(bias_g),
        })
    return in_maps


def _gather(results):
    out_full = np.empty((B, N, DIM), dtype=np.float32)
    for c in range(NCORES):
        b, g = divmod(c, 2)
        oc = results[c]["out"].astype(np.float32)  # [NPAIR, 128, N] bf16
        for p in range(NPAIR):
            for u in range(2):
                h = 2 * p + u
                col = g * GD + h * DK
                out_full[b, :, col:col + DK] = oc[p, u * 64:(u + 1) * 64, :].T
    return out_full


def kernel(x, Wq, Wk, rel_content_bias):
    x = np.asarray(x, dtype=np.float32)
    Wq = np.asarray(Wq, dtype=np.float32)
    Wk = np.asarray(Wk, dtype=np.float32)
    rb = np.asarray(rel_content_bias, dtype=np.float32)

    nc = _get_nc()
    in_maps = _make_in_maps(x, Wq, Wk, rb)
    res = run_bass_kernel_spmd(nc, in_maps, core_ids=list(range(NCORES)))
    return _gather(res.results)
